# revision 8
# baseline (speedup 1.0000x reference)
"""Cost-volume kernel for Trainium2 (8 NeuronCores, Bass).

cost[b, i, h, w] = mean_c f1[b,c,h,w] * f2[b,c,h,w-i]  (0 where w < i)

Per (b, h) plane (C=128 on partitions), fp16 datapath / fp32 accumulation:
  f2r[c, v] = fp16(f2[c, 255-v]), zeros for v in [256, 320)   (DVE reverse+cast)
  H2[w, v]  = sum_c f1[c, w] * f2r[c, v]      (PE fp16, 2 matmul tiles, fp32 PSUM)
  hc        = fp16(H2)                        (ACT/DVE copy PSUM->SBUF)
  band: out[j, w] = H2[w, 255-w+j]            (ONE anti-diagonal DMA per pair:
                                               src steps [767, 192, 1] -- covers
                                               both w-halves x both planes)
  PE transpose (PK^T @ (I/128)) -> Tt[j, w] = output plane (fp32, scale folded)
  copy PSUM->SBUF (DVE/ACT parity split); DMA out (fp32).

Granularity: compute stages per plane-PAIR; DMA stages per QUAD (4 planes).
Stage-lagged software pipeline; per-buffer-slot DMA semaphores.  DMA rings:
  Pool/SWDGE (gpsimd): f1 quad loads with fp32->fp16 cast, quad memsets
  ACT ring:            f2 quad loads (fp32)
  SP ring:             fused shear (per pair) + quad output stores (fp32)

Sharding: 8 cores x 16 H-rows (data-parallel over B*H planes, 64 planes/core).
"""
import numpy as np

import concourse.bass as bass
import concourse.mybir as mybir
from concourse.bass_utils import run_bass_kernel_spmd

B, C, H, W = 4, 128, 128, 256
L = 64
NCORES = 8
HS = H // NCORES          # 16 h-rows per core
NPL = B * HS              # 64 planes per core
NPR = NPL // 2            # 32 pairs per core
NQ = NPR // 2             # 16 quads per core

# stage lags in pair-iterations (quad stages fire on matching parity)
LAG_REV = 1
LAG_MM = 2
LAG_HC = 3
LAG_SH = 4
LAG_TT = 5
LAG_T2 = 6
LAG_OUT = 7
NIT = NPR + 8

NBQ = 2           # F1/F2/F2R quad buffers (4 planes each)
NHC = 3           # HC pair buffers
NPK = 3           # PK pair buffers
NT2Q = 2          # T2 quad buffers
NPH = 2           # PSUM pair slots for H2
NPT = 2           # PSUM pair slots for transpose out

F32 = mybir.dt.float32
F16 = mybir.dt.float16


def _build(nc_holder={}):
    if "nc" in nc_holder:
        return nc_holder["nc"]
    nc = bass.Bass()
    f1 = nc.dram_tensor("f1", [B, C, HS, W], F32, kind="ExternalInput")
    f2 = nc.dram_tensor("f2", [B, C, HS, W], F32, kind="ExternalInput")
    ident = nc.dram_tensor("ident", [128, 128], F16, kind="ExternalInput")
    out = nc.dram_tensor("out", [B, L, HS, W], F32, kind="ExternalOutput")

    from contextlib import ExitStack
    ctx = ExitStack()
    sem = lambda n: ctx.enter_context(nc.semaphore(n))
    sbuf = lambda n, s, dt: ctx.enter_context(nc.sbuf_tensor(n, s, dt))
    psum = lambda n, s: ctx.enter_context(nc.psum_tensor(n, s, F32))

    sI = sem("sI")
    sF1 = [sem(f"sF1_{k}") for k in range(NBQ)]
    sF2 = [sem(f"sF2_{k}") for k in range(NBQ)]
    sSh = [sem(f"sSh_{k}") for k in range(NPK)]
    sO = [sem(f"sO_{k}") for k in range(NT2Q)]
    cR = sem("cR")     # revcopy, +1/quad
    cZ = sem("cZ")     # memset, +1/quad
    cM = sem("cM")     # gram mms, +4/pair
    cHe = sem("cHe")   # HC copy even pairs (ACT), +1
    cHo = sem("cHo")   # HC copy odd pairs (DVE), +1
    cT = sem("cT")     # transposes, +4/pair
    cVe = sem("cVe")   # T2 copy even pairs (DVE), +1
    cVo = sem("cVo")   # T2 copy odd pairs (ACT), +1

    I = sbuf("I", [128, 128], F16)
    F1Q = [sbuf(f"F1Q_{k}", [128, 1024], F16) for k in range(NBQ)]
    F2Q = [sbuf(f"F2Q_{k}", [128, 1024], F32) for k in range(NBQ)]
    F2R = [sbuf(f"F2R_{k}", [128, 1280], F16) for k in range(NBQ)]
    HC = [sbuf(f"HC_{k}", [128, 768], F16) for k in range(NHC)]
    PK = [sbuf(f"PK_{k}", [128, 256], F16) for k in range(NPK)]
    T2 = [sbuf(f"T2_{k}", [64, 1024], F32) for k in range(NT2Q)]
    Hp = [psum(f"Hp_{k}", [128, 1024]) for k in range(NPH)]
    Tt = [psum(f"Tt_{k}", [64, 512]) for k in range(NPT)]

    uses = lambda t, n: 16 * (t // n + 1)

    def quad_base(t):
        b, hl = (4 * t) // HS, (4 * t) % HS
        return b, hl

    def f1_quad(t):
        b, hl = quad_base(t)
        return bass.AP(f1, (b * C * HS + hl) * W, [[HS * W, 128], [W, 4], [1, W]])

    def f2_quad(t):
        b, hl = quad_base(t)
        return bass.AP(f2, (b * C * HS + hl) * W, [[HS * W, 128], [W, 4], [1, W]])

    def out_quad(t):
        b, hl = quad_base(t)
        return bass.AP(out, (b * L * HS + hl) * W, [[HS * W, 64], [W, 4], [1, W]])

    def wait_hc(engine, q):
        if q % 2 == 0:
            engine.wait_ge(cHe, q // 2 + 1)
        else:
            engine.wait_ge(cHo, q // 2 + 1)

    def wait_t2(engine, q):
        if q % 2 == 0:
            engine.wait_ge(cVe, q // 2 + 1)
        else:
            engine.wait_ge(cVo, q // 2 + 1)

    def hc_copy(engine, q):
        # HC(q) <- fp16(Hp(q)); Hp pair: planes at cols [0:384) and [512:896)
        engine.wait_ge(cM, 4 * (q + 1))
        if q >= NHC:
            qq = q - NHC
            engine.wait_ge(sSh[qq % NPK], uses(qq, NPK))   # HC slot free
        copy_fn = getattr(engine, "tensor_copy", None) or engine.copy
        copy_fn(
            bass.AP(HC[q % NHC], 0, [[768, 128], [384, 2], [1, 384]]),
            bass.AP(Hp[q % NPH], 0, [[1024, 128], [512, 2], [1, 384]]),
        ).then_inc(cHe if q % 2 == 0 else cHo, 1)

    def t2_copy(engine, q):
        # T2 quad slot (q//2) % NT2Q, half q%2  <-  Tt[q % NPT]
        t = q // 2
        if t >= NT2Q:
            tt_ = t - NT2Q
            engine.wait_ge(sO[tt_ % NT2Q], uses(tt_, NT2Q))  # T2 slot free
        engine.wait_ge(cT, 4 * (q + 1))                      # transposes(q) done
        copy_fn = getattr(engine, "tensor_copy", None) or engine.copy
        copy_fn(
            bass.AP(T2[t % NT2Q], 512 * (q % 2), [[1024, 64], [1, 512]]),
            Tt[q % NPT][:, :],
        ).then_inc(cVe if q % 2 == 0 else cVo, 1)

    with nc.Block() as block:

        @block.sync
        def _(sync):
            sync.dma_start(I[:, :], ident[:, :]).then_inc(sI, 16)
            for i in range(NIT):
                q = i - LAG_SH
                if 0 <= q < NPR:
                    wait_hc(sync, q)                         # HC(q) written
                    if q >= NPK:
                        sync.wait_ge(cT, 4 * (q - NPK + 1))  # PK slot free
                    # fused shear: both halves x both planes, steps [767, 192, 1]
                    sync.dma_start(
                        bass.AP(PK[q % NPK], 0, [[256, 128], [64, 4], [1, 64]]),
                        bass.AP(HC[q % NHC], 127, [[767, 128], [192, 4], [1, 64]]),
                    ).then_inc(sSh[q % NPK], 16)
                j = i - LAG_OUT
                if 0 <= j < NPR and j % 2 == 1:
                    t = (j - 1) // 2
                    sync.wait_ge(cVe, t + 1)                 # T2 even half (pair 2t)
                    sync.wait_ge(cVo, t + 1)                 # T2 odd half (pair 2t+1)
                    sync.dma_start(
                        out_quad(t),
                        bass.AP(T2[t % NT2Q], 0, [[1024, 64], [256, 4], [1, 256]]),
                    ).then_inc(sO[t % NT2Q], 16)

        @block.scalar
        def _(scalar):
            for i in range(NIT):
                if i % 2 == 0 and i // 2 < NQ:
                    t = i // 2
                    if t >= NBQ:
                        scalar.wait_ge(cR, t - NBQ + 1)      # F2 slot free
                    scalar.dma_start(F2Q[t % NBQ][:, :], f2_quad(t)).then_inc(sF2[t % NBQ], 16)
                q = i - LAG_HC
                if 0 <= q < NPR and q % 2 == 0:
                    hc_copy(scalar, q)
                q = i - LAG_T2
                if 0 <= q < NPR and q % 2 == 1:
                    t2_copy(scalar, q)

        @block.gpsimd
        def _(gpsimd):
            for i in range(NIT):
                if i % 2 == 0 and i // 2 < NQ:
                    t = i // 2
                    if t >= NBQ:
                        gpsimd.wait_ge(cM, 8 * (t - NBQ + 1))  # F1/F2R quad slot free
                    gpsimd.dma_start(
                        bass.AP(F1Q[t % NBQ], 0, [[1024, 128], [256, 4], [1, 256]]),
                        f1_quad(t),
                    ).then_inc(sF1[t % NBQ], 16)
                    gpsimd.memset(
                        bass.AP(F2R[t % NBQ], 256, [[1280, 128], [320, 4], [1, 64]]), 0.0
                    ).then_inc(cZ, 1)

        @block.vector
        def _(vector):
            for i in range(NIT):
                if i % 2 == 1 and i // 2 < NQ:
                    t = i // 2                                # revcopy quad t
                    if t >= NBQ:
                        vector.wait_ge(cM, 8 * (t - NBQ + 1))  # F2R slot free
                    vector.wait_ge(cZ, t + 1)
                    vector.wait_ge(sF2[t % NBQ], uses(t, NBQ))
                    vector.tensor_copy(
                        bass.AP(F2R[t % NBQ], 0, [[1280, 128], [320, 4], [1, 256]]),
                        bass.AP(F2Q[t % NBQ], 255, [[1024, 128], [256, 4], [-1, 256]]),
                    ).then_inc(cR, 1)
                q = i - LAG_HC
                if 0 <= q < NPR and q % 2 == 1:
                    hc_copy(vector, q)
                q = i - LAG_T2
                if 0 <= q < NPR and q % 2 == 0:
                    t2_copy(vector, q)

        @block.tensor
        def _(tensor):
            for i in range(NIT):
                q = i - LAG_MM
                if 0 <= q < NPR:
                    t, r = q // 2, q % 2
                    tensor.wait_ge(sF1[t % NBQ], uses(t, NBQ))  # F1 quad loaded
                    tensor.wait_ge(cR, t + 1)                   # F2R quad ready
                    if q >= NPH:
                        wait_hc(tensor, q - NPH)                # Hp slot free
                    hp = Hp[q % NPH]
                    f1o, f2o = 512 * r, 640 * r
                    f1t, f2r = F1Q[t % NBQ], F2R[t % NBQ]
                    tensor.matmul(hp[:, 0:192], f1t[:, f1o:f1o + 128],
                                  f2r[:, f2o + 128:f2o + 320]).then_inc(cM, 1)
                    tensor.matmul(hp[:, 192:384], f1t[:, f1o + 128:f1o + 256],
                                  f2r[:, f2o:f2o + 192]).then_inc(cM, 1)
                    tensor.matmul(hp[:, 512:704], f1t[:, f1o + 256:f1o + 384],
                                  f2r[:, f2o + 448:f2o + 640]).then_inc(cM, 1)
                    tensor.matmul(hp[:, 704:896], f1t[:, f1o + 384:f1o + 512],
                                  f2r[:, f2o + 320:f2o + 512]).then_inc(cM, 1)
                q = i - LAG_TT
                if 0 <= q < NPR:
                    if q == 0:
                        tensor.wait_ge(sI, 16)
                    if q >= NPT:
                        wait_t2(tensor, q - NPT)                # Tt slot free
                    tensor.wait_ge(sSh[q % NPK], uses(q, NPK))  # shear(q) done
                    tt, pk = Tt[q % NPT], PK[q % NPK]
                    tensor.matmul(tt[:, 0:128], pk[:, 0:64], I[:, :]).then_inc(cT, 1)
                    tensor.matmul(tt[:, 128:256], pk[:, 64:128], I[:, :]).then_inc(cT, 1)
                    tensor.matmul(tt[:, 256:384], pk[:, 128:192], I[:, :]).then_inc(cT, 1)
                    tensor.matmul(tt[:, 384:512], pk[:, 192:256], I[:, :]).then_inc(cT, 1)

    nc_holder["nc"] = nc
    return nc


def run_sharded(features_1: np.ndarray, features_2: np.ndarray, **spmd_kwargs):
    """Shard over H, run on 8 cores, return (full_output, BassKernelResults)."""
    nc = _build()
    ident = (np.eye(128, dtype=np.float32) / 128.0).astype(np.float16)
    in_maps = []
    for k in range(NCORES):
        sl = slice(k * HS, (k + 1) * HS)
        in_maps.append({
            "f1": np.ascontiguousarray(features_1[:, :, sl, :], dtype=np.float32),
            "f2": np.ascontiguousarray(features_2[:, :, sl, :], dtype=np.float32),
            "ident": ident,
        })
    res = run_bass_kernel_spmd(nc, in_maps, core_ids=list(range(NCORES)), **spmd_kwargs)
    full = np.empty((B, L, H, W), dtype=np.float32)
    for k in range(NCORES):
        full[:, :, k * HS:(k + 1) * HS, :] = res.results[k]["out"]
    return full, res


def kernel(features_1, features_2, lvls) -> np.ndarray:
    assert int(lvls) == L
    f1 = np.asarray(features_1, dtype=np.float32)
    f2 = np.asarray(features_2, dtype=np.float32)
    full, _ = run_sharded(f1, f2)
    return full


# revision 9
# speedup vs baseline: 1.1465x; 1.1465x over previous
"""Cost-volume kernel for Trainium2 (8 NeuronCores, Bass).

cost[b, i, h, w] = mean_c f1[b,c,h,w] * f2[b,c,h,w-i]  (0 where w < i)

Per (b, h) plane (C=128 on partitions), fp16 datapath / fp32 accumulation:
  f2r[c, v] = fp16(f2[c, 255-v]), zeros for v in [256, 320)   (DVE reverse+cast)
  H2[w, v]  = sum_c f1[c, w] * f2r[c, v]      (PE fp16, 2 matmul tiles, fp32 PSUM)
  hc        = fp16(H2)                        (ACT/DVE copy PSUM->SBUF)
  band: out[j, w] = H2[w, 255-w+j]            (ONE anti-diagonal DMA per pair:
                                               src steps [767, 192, 1] -- covers
                                               both w-halves x both planes)
  PE transpose (PK^T @ (I/128)) -> Tt[j, w] = output plane (fp32, scale folded)
  copy PSUM->SBUF (DVE/ACT parity split); DMA out (fp32).

Granularity: compute stages per plane-PAIR; DMA stages per QUAD (4 planes).
Stage-lagged software pipeline; per-buffer-slot DMA semaphores.  DMA rings:
  Pool/SWDGE (gpsimd): f1 quad loads with fp32->fp16 cast, quad memsets
  ACT ring:            f2 quad loads (fp32)
  SP ring:             fused shear (per pair) + quad output stores (fp32)

Sharding: 8 cores x 16 H-rows (data-parallel over B*H planes, 64 planes/core).
"""
import numpy as np

import concourse.bass as bass
import concourse.mybir as mybir
from concourse.bass_utils import run_bass_kernel_spmd

B, C, H, W = 4, 128, 128, 256
L = 64
NCORES = 8
HS = H // NCORES          # 16 h-rows per core
NPL = B * HS              # 64 planes per core
NPR = NPL // 2            # 32 pairs per core
NQ = NPR // 2             # 16 quads per core

# stage lags in pair-iterations (quad stages fire on matching parity).
# Every DMA-flight edge gets >=3 iterations of slack so transfer+completion
# latency (~2us) never sits on the steady-state critical path.
LAG_REVQ = 3      # revcopy of quad t fires at iteration 2t+3
LAG_MM = 5
LAG_HC = 6
LAG_SH = 7
LAG_TT = 10
LAG_T2 = 11
LAG_OUT = 14      # out of quad t fires at iteration 2t+14
NIT = NPR + 17

NBQ = 4           # F1/F2/F2R quad buffers (4 planes each)
NHC = 4           # HC pair buffers
NPK = 4           # PK pair buffers
NT2Q = 3          # T2 quad buffers
NPH = 3           # PSUM pair slots for H2 (2 banks each)
NPT = 2           # PSUM pair slots for transpose out (1 bank each)

F32 = mybir.dt.float32
F16 = mybir.dt.float16


def _build(nc_holder={}):
    if "nc" in nc_holder:
        return nc_holder["nc"]
    nc = bass.Bass()
    f1 = nc.dram_tensor("f1", [B, C, HS, W], F32, kind="ExternalInput")
    f2 = nc.dram_tensor("f2", [B, C, HS, W], F32, kind="ExternalInput")
    ident = nc.dram_tensor("ident", [128, 128], F16, kind="ExternalInput")
    out = nc.dram_tensor("out", [B, L, HS, W], F32, kind="ExternalOutput")

    from contextlib import ExitStack
    ctx = ExitStack()
    sem = lambda n: ctx.enter_context(nc.semaphore(n))
    sbuf = lambda n, s, dt: ctx.enter_context(nc.sbuf_tensor(n, s, dt))
    psum = lambda n, s: ctx.enter_context(nc.psum_tensor(n, s, F32))

    sI = sem("sI")
    sF1 = [sem(f"sF1_{k}") for k in range(NBQ)]
    sF2 = [sem(f"sF2_{k}") for k in range(NBQ)]
    sSh = [sem(f"sSh_{k}") for k in range(NPK)]
    sO = [sem(f"sO_{k}") for k in range(NT2Q)]
    cR = sem("cR")     # revcopy, +1/quad
    cZ = sem("cZ")     # memset, +1/quad
    cM = sem("cM")     # gram mms, +4/pair
    cHe = sem("cHe")   # HC copy even pairs (ACT), +1
    cHo = sem("cHo")   # HC copy odd pairs (DVE), +1
    cT = sem("cT")     # transposes, +4/pair
    cVe = sem("cVe")   # T2 copy even pairs (DVE), +1
    cVo = sem("cVo")   # T2 copy odd pairs (ACT), +1

    I = sbuf("I", [128, 128], F16)
    F1Q = [sbuf(f"F1Q_{k}", [128, 1024], F16) for k in range(NBQ)]
    F2Q = [sbuf(f"F2Q_{k}", [128, 1024], F32) for k in range(NBQ)]
    F2R = [sbuf(f"F2R_{k}", [128, 1280], F16) for k in range(NBQ)]
    HC = [sbuf(f"HC_{k}", [128, 768], F16) for k in range(NHC)]
    PK = [sbuf(f"PK_{k}", [128, 256], F16) for k in range(NPK)]
    T2 = [sbuf(f"T2_{k}", [64, 1024], F32) for k in range(NT2Q)]
    Hp = [psum(f"Hp_{k}", [128, 1024]) for k in range(NPH)]
    Tt = [psum(f"Tt_{k}", [64, 512]) for k in range(NPT)]

    uses = lambda t, n: 16 * (t // n + 1)

    def quad_base(t):
        b, hl = (4 * t) // HS, (4 * t) % HS
        return b, hl

    def f1_quad(t):
        b, hl = quad_base(t)
        return bass.AP(f1, (b * C * HS + hl) * W, [[HS * W, 128], [W, 4], [1, W]])

    def f2_quad(t):
        b, hl = quad_base(t)
        return bass.AP(f2, (b * C * HS + hl) * W, [[HS * W, 128], [W, 4], [1, W]])

    def out_quad(t):
        b, hl = quad_base(t)
        return bass.AP(out, (b * L * HS + hl) * W, [[HS * W, 64], [W, 4], [1, W]])

    def wait_hc(engine, q):
        if q % 2 == 0:
            engine.wait_ge(cHe, q // 2 + 1)
        else:
            engine.wait_ge(cHo, q // 2 + 1)

    def wait_t2(engine, q):
        if q % 2 == 0:
            engine.wait_ge(cVe, q // 2 + 1)
        else:
            engine.wait_ge(cVo, q // 2 + 1)

    def hc_copy(engine, q):
        # HC(q) <- fp16(Hp(q)); Hp pair: planes at cols [0:384) and [512:896)
        engine.wait_ge(cM, 4 * (q + 1))
        if q >= NHC:
            qq = q - NHC
            engine.wait_ge(sSh[qq % NPK], uses(qq, NPK))   # HC slot free
        copy_fn = getattr(engine, "tensor_copy", None) or engine.copy
        copy_fn(
            bass.AP(HC[q % NHC], 0, [[768, 128], [384, 2], [1, 384]]),
            bass.AP(Hp[q % NPH], 0, [[1024, 128], [512, 2], [1, 384]]),
        ).then_inc(cHe if q % 2 == 0 else cHo, 1)

    def t2_copy(engine, q):
        # T2 quad slot (q//2) % NT2Q, half q%2  <-  Tt[q % NPT]
        t = q // 2
        if t >= NT2Q:
            tt_ = t - NT2Q
            engine.wait_ge(sO[tt_ % NT2Q], uses(tt_, NT2Q))  # T2 slot free
        engine.wait_ge(cT, 4 * (q + 1))                      # transposes(q) done
        copy_fn = getattr(engine, "tensor_copy", None) or engine.copy
        copy_fn(
            bass.AP(T2[t % NT2Q], 512 * (q % 2), [[1024, 64], [1, 512]]),
            Tt[q % NPT][:, :],
        ).then_inc(cVe if q % 2 == 0 else cVo, 1)

    with nc.Block() as block:

        @block.sync
        def _(sync):
            sync.dma_start(I[:, :], ident[:, :]).then_inc(sI, 16)
            for i in range(NIT):
                q = i - LAG_SH
                if 0 <= q < NPR:
                    wait_hc(sync, q)                         # HC(q) written
                    if q >= NPK:
                        sync.wait_ge(cT, 4 * (q - NPK + 1))  # PK slot free
                    # fused shear: both halves x both planes, steps [767, 192, 1]
                    sync.dma_start(
                        bass.AP(PK[q % NPK], 0, [[256, 128], [64, 4], [1, 64]]),
                        bass.AP(HC[q % NHC], 127, [[767, 128], [192, 4], [1, 64]]),
                    ).then_inc(sSh[q % NPK], 16)
                j = i - LAG_OUT
                if 0 <= j < NPR and j % 2 == 0:
                    t = j // 2
                    sync.wait_ge(cVe, t + 1)                 # T2 even half (pair 2t)
                    sync.wait_ge(cVo, t + 1)                 # T2 odd half (pair 2t+1)
                    sync.dma_start(
                        out_quad(t),
                        bass.AP(T2[t % NT2Q], 0, [[1024, 64], [256, 4], [1, 256]]),
                    ).then_inc(sO[t % NT2Q], 16)

        @block.scalar
        def _(scalar):
            for i in range(NIT):
                if i % 2 == 0 and i // 2 < NQ:
                    t = i // 2
                    if t >= NBQ:
                        scalar.wait_ge(cR, t - NBQ + 1)      # F2 slot free
                    scalar.dma_start(F2Q[t % NBQ][:, :], f2_quad(t)).then_inc(sF2[t % NBQ], 16)
                q = i - LAG_HC
                if 0 <= q < NPR and q % 2 == 0:
                    hc_copy(scalar, q)
                q = i - LAG_T2
                if 0 <= q < NPR and q % 2 == 1:
                    t2_copy(scalar, q)

        @block.gpsimd
        def _(gpsimd):
            for i in range(NIT):
                if i % 2 == 0 and i // 2 < NQ:
                    t = i // 2
                    if t >= NBQ:
                        gpsimd.wait_ge(cM, 8 * (t - NBQ + 1))  # F1/F2R quad slot free
                    gpsimd.dma_start(
                        bass.AP(F1Q[t % NBQ], 0, [[1024, 128], [256, 4], [1, 256]]),
                        f1_quad(t),
                    ).then_inc(sF1[t % NBQ], 16)
                    gpsimd.memset(
                        bass.AP(F2R[t % NBQ], 256, [[1280, 128], [320, 4], [1, 64]]), 0.0
                    ).then_inc(cZ, 1)

        @block.vector
        def _(vector):
            for i in range(NIT):
                if i % 2 == 1 and (i - LAG_REVQ) % 2 == 0 and 0 <= (i - LAG_REVQ) // 2 < NQ:
                    t = (i - LAG_REVQ) // 2                   # revcopy quad t
                    if t >= NBQ:
                        vector.wait_ge(cM, 8 * (t - NBQ + 1))  # F2R slot free
                    vector.wait_ge(cZ, t + 1)
                    vector.wait_ge(sF2[t % NBQ], uses(t, NBQ))
                    vector.tensor_copy(
                        bass.AP(F2R[t % NBQ], 0, [[1280, 128], [320, 4], [1, 256]]),
                        bass.AP(F2Q[t % NBQ], 255, [[1024, 128], [256, 4], [-1, 256]]),
                    ).then_inc(cR, 1)
                q = i - LAG_HC
                if 0 <= q < NPR and q % 2 == 1:
                    hc_copy(vector, q)
                q = i - LAG_T2
                if 0 <= q < NPR and q % 2 == 0:
                    t2_copy(vector, q)

        @block.tensor
        def _(tensor):
            for i in range(NIT):
                q = i - LAG_MM
                if 0 <= q < NPR:
                    t, r = q // 2, q % 2
                    tensor.wait_ge(sF1[t % NBQ], uses(t, NBQ))  # F1 quad loaded
                    tensor.wait_ge(cR, t + 1)                   # F2R quad ready
                    if q >= NPH:
                        wait_hc(tensor, q - NPH)                # Hp slot free
                    hp = Hp[q % NPH]
                    f1o, f2o = 512 * r, 640 * r
                    f1t, f2r = F1Q[t % NBQ], F2R[t % NBQ]
                    tensor.matmul(hp[:, 0:192], f1t[:, f1o:f1o + 128],
                                  f2r[:, f2o + 128:f2o + 320]).then_inc(cM, 1)
                    tensor.matmul(hp[:, 192:384], f1t[:, f1o + 128:f1o + 256],
                                  f2r[:, f2o:f2o + 192]).then_inc(cM, 1)
                    tensor.matmul(hp[:, 512:704], f1t[:, f1o + 256:f1o + 384],
                                  f2r[:, f2o + 448:f2o + 640]).then_inc(cM, 1)
                    tensor.matmul(hp[:, 704:896], f1t[:, f1o + 384:f1o + 512],
                                  f2r[:, f2o + 320:f2o + 512]).then_inc(cM, 1)
                q = i - LAG_TT
                if 0 <= q < NPR:
                    if q == 0:
                        tensor.wait_ge(sI, 16)
                    if q >= NPT:
                        wait_t2(tensor, q - NPT)                # Tt slot free
                    tensor.wait_ge(sSh[q % NPK], uses(q, NPK))  # shear(q) done
                    tt, pk = Tt[q % NPT], PK[q % NPK]
                    tensor.matmul(tt[:, 0:128], pk[:, 0:64], I[:, :]).then_inc(cT, 1)
                    tensor.matmul(tt[:, 128:256], pk[:, 64:128], I[:, :]).then_inc(cT, 1)
                    tensor.matmul(tt[:, 256:384], pk[:, 128:192], I[:, :]).then_inc(cT, 1)
                    tensor.matmul(tt[:, 384:512], pk[:, 192:256], I[:, :]).then_inc(cT, 1)

    nc_holder["nc"] = nc
    return nc


def run_sharded(features_1: np.ndarray, features_2: np.ndarray, **spmd_kwargs):
    """Shard over H, run on 8 cores, return (full_output, BassKernelResults)."""
    nc = _build()
    ident = (np.eye(128, dtype=np.float32) / 128.0).astype(np.float16)
    in_maps = []
    for k in range(NCORES):
        sl = slice(k * HS, (k + 1) * HS)
        in_maps.append({
            "f1": np.ascontiguousarray(features_1[:, :, sl, :], dtype=np.float32),
            "f2": np.ascontiguousarray(features_2[:, :, sl, :], dtype=np.float32),
            "ident": ident,
        })
    res = run_bass_kernel_spmd(nc, in_maps, core_ids=list(range(NCORES)), **spmd_kwargs)
    full = np.empty((B, L, H, W), dtype=np.float32)
    for k in range(NCORES):
        full[:, :, k * HS:(k + 1) * HS, :] = res.results[k]["out"]
    return full, res


def kernel(features_1, features_2, lvls) -> np.ndarray:
    assert int(lvls) == L
    f1 = np.asarray(features_1, dtype=np.float32)
    f2 = np.asarray(features_2, dtype=np.float32)
    full, _ = run_sharded(f1, f2)
    return full
